# revision 6
# baseline (speedup 1.0000x reference)
"""Trainium2 Bass kernel for a 12-layer BERT encoder + ragged segment-mean pooling.

Sharding: data-parallel over batch — 8 sequences, one per NeuronCore. Each core
runs the full encoder on its [256, 768] activation with replicated weights
(streamed from HBM in bf16), then pools 256 subwords -> 128 tokens with a
host-precomputed pooling matrix applied on the tensor engine.

Layout strategy per core:
  - residual stream h/t/h2 kept token-major [128part, 2chunk, 768] in fp32
    (LayerNorm reduces along the free dim; bn_stats/bn_aggr)
  - matmul operands in bf16; activations transposed via PE transpose where a
    matmul needs them as lhsT/rhs ([hdim-part, seq-free])
  - attention computed in "scoresT" orientation (keys on partitions, queries on
    free dim): the additive sequence mask becomes a per-partition bias folded
    into the Exp activation, and the softmax denominator is a PE ones-matmul
    column sum, broadcast back across partitions with a second tiny matmul.
"""

import numpy as np
import ml_dtypes

H = 768
L = 12
NH = 12
HD = 64
FF = 3072
VOCAB = 30522
BZ = 8
S = 256
T = 128
EPS = 1e-12
P = 128
NCHUNK = S // P  # 2 token chunks of 128
KT = H // P      # 6 contraction tiles over hidden dim
FKT = FF // P    # 24 contraction tiles over ffn dim

BF16 = ml_dtypes.bfloat16

_CACHE = {}


def _build_program(flags):
    import concourse.bacc as bacc
    import concourse.mybir as mybir
    import concourse.tile as tile
    from concourse.masks import make_identity

    dt = mybir.dt
    AF = mybir.ActivationFunctionType
    OP = mybir.AluOpType

    nc = bacc.Bacc(
        "TRN2",
        target_bir_lowering=False,
        debug=False,
        enable_asserts=False,
        num_devices=8,
    )

    # ---- DRAM I/O ----
    ids_d = nc.dram_tensor("ids32", (NCHUNK, P), dt.int32, kind="ExternalInput").ap()
    seg_d = nc.dram_tensor("seg32", (NCHUNK, P), dt.int32, kind="ExternalInput").ap()
    am_d = nc.dram_tensor("am", (P, NCHUNK), dt.float32, kind="ExternalInput").ap()
    atp_d = nc.dram_tensor("atp", (P, NCHUNK, T), dt.float32, kind="ExternalInput").ap()
    wemb_d = nc.dram_tensor("word_emb", (VOCAB, H), dt.float32, kind="ExternalInput").ap()
    pemb_d = nc.dram_tensor("pos_emb", (S, H), dt.float32, kind="ExternalInput").ap()
    temb_d = nc.dram_tensor("type_emb", (2, H), dt.float32, kind="ExternalInput").ap()
    qkvw_d = nc.dram_tensor("qkv_w", (L, H, 3 * H), dt.bfloat16, kind="ExternalInput").ap()
    ow_d = nc.dram_tensor("attn_out_w", (L, H, H), dt.bfloat16, kind="ExternalInput").ap()
    f1w_d = nc.dram_tensor("ff1_w", (L, H, FF), dt.bfloat16, kind="ExternalInput").ap()
    f2w_d = nc.dram_tensor("ff2_w", (L, FF, H), dt.bfloat16, kind="ExternalInput").ap()

    # optional (only present when the corresponding values are nontrivial)
    gb_d = {}
    if flags["emb_gb"]:
        gb_d["emb_g"] = nc.dram_tensor("emb_g", (H,), dt.float32, kind="ExternalInput").ap()
        gb_d["emb_b"] = nc.dram_tensor("emb_b", (H,), dt.float32, kind="ExternalInput").ap()
    if flags["ln1_gb"]:
        gb_d["ln1_g"] = nc.dram_tensor("ln1_g", (L, H), dt.float32, kind="ExternalInput").ap()
        gb_d["ln1_b"] = nc.dram_tensor("ln1_b", (L, H), dt.float32, kind="ExternalInput").ap()
    if flags["ln2_gb"]:
        gb_d["ln2_g"] = nc.dram_tensor("ln2_g", (L, H), dt.float32, kind="ExternalInput").ap()
        gb_d["ln2_b"] = nc.dram_tensor("ln2_b", (L, H), dt.float32, kind="ExternalInput").ap()
    if flags["qkv_b"]:
        # host pre-scales the q third by 1/sqrt(HD)
        gb_d["qkv_b"] = nc.dram_tensor("qkv_b", (L, 3 * H), dt.float32, kind="ExternalInput").ap()
    if flags["ob"]:
        gb_d["ob"] = nc.dram_tensor("ob", (L, H), dt.float32, kind="ExternalInput").ap()
    if flags["f1b"]:
        gb_d["f1b"] = nc.dram_tensor("f1b", (L, FF), dt.float32, kind="ExternalInput").ap()
    if flags["f2b"]:
        gb_d["f2b"] = nc.dram_tensor("f2b", (L, H), dt.float32, kind="ExternalInput").ap()

    out_d = nc.dram_tensor("outp", (T, H), dt.float32, kind="ExternalOutput").ap()

    with tile.TileContext(nc) as tc:
        from contextlib import ExitStack

        ctx = ExitStack()
        with ctx:
            persist = ctx.enter_context(tc.tile_pool(name="persist", bufs=1))
            wpool = ctx.enter_context(tc.tile_pool(name="wpool", bufs=4))
            wbig = ctx.enter_context(tc.tile_pool(name="wbig", bufs=2))
            work = ctx.enter_context(tc.tile_pool(name="work", bufs=3))
            ps = ctx.enter_context(tc.tile_pool(name="ps", bufs=2, space="PSUM"))

            # ---- persistent tiles ----
            ident = persist.tile([P, P], dt.float32, tag="ident")
            make_identity(nc, ident)
            ones_bf = persist.tile([P, 1], dt.bfloat16, tag="ones_bf")
            nc.vector.memset(ones_bf, 1.0)
            ones1 = persist.tile([1, HD], dt.float32, tag="ones1")
            nc.vector.memset(ones1, 1.0)
            am_sb = persist.tile([P, NCHUNK], dt.float32, tag="am_sb")
            nc.sync.dma_start(am_sb, am_d)
            eps_sb = persist.tile([P, 1], dt.float32, tag="eps_sb")
            nc.vector.memset(eps_sb, EPS)
            atp_sb = persist.tile([P, NCHUNK, T], dt.float32, tag="atp_sb")
            nc.sync.dma_start(atp_sb, atp_d)

            h = persist.tile([P, NCHUNK, H], dt.float32, tag="h")
            t = persist.tile([P, NCHUNK, H], dt.float32, tag="t")
            h2 = persist.tile([P, NCHUNK, H], dt.float32, tag="h2")
            hT = persist.tile([P, KT, S], dt.bfloat16, tag="hT")
            h2T = persist.tile([P, KT, S], dt.bfloat16, tag="h2T")
            qT = persist.tile([P, KT, S], dt.bfloat16, tag="qT")
            kTt = persist.tile([P, KT, S], dt.bfloat16, tag="kTt")
            vtm = persist.tile([P, NCHUNK, H], dt.bfloat16, tag="vtm")
            ctxT = persist.tile([P, KT, S], dt.bfloat16, tag="ctxT")
            fT = persist.tile([P, FKT, S], dt.bfloat16, tag="fT")
            out_sb = persist.tile([P, H], dt.float32, tag="out_sb")

            # broadcast (across partitions) gain/bias tiles, if nontrivial
            def bcast_load(name, src_ap, width):
                til = persist.tile([P, width], dt.float32, tag=name, name=name)
                import concourse.bass as bass

                bap = bass.AP(
                    tensor=src_ap.tensor,
                    offset=src_ap.offset,
                    ap=[[0, P]] + list(src_ap.ap),
                )
                nc.gpsimd.dma_start(out=til, in_=bap)
                return til

            gb_sb = {}
            if flags["emb_gb"]:
                gb_sb["emb_g"] = bcast_load("emb_g_sb", gb_d["emb_g"], H)
                gb_sb["emb_b"] = bcast_load("emb_b_sb", gb_d["emb_b"], H)
            if flags["ln1_gb"]:
                gb_sb["ln1_g"] = bcast_load("ln1_g_sb", gb_d["ln1_g"].rearrange("l h -> (l h)"), L * H)
                gb_sb["ln1_b"] = bcast_load("ln1_b_sb", gb_d["ln1_b"].rearrange("l h -> (l h)"), L * H)
            if flags["ln2_gb"]:
                gb_sb["ln2_g"] = bcast_load("ln2_g_sb", gb_d["ln2_g"].rearrange("l h -> (l h)"), L * H)
                gb_sb["ln2_b"] = bcast_load("ln2_b_sb", gb_d["ln2_b"].rearrange("l h -> (l h)"), L * H)
            if flags["ob"]:
                gb_sb["ob"] = bcast_load("ob_sb", gb_d["ob"].rearrange("l h -> (l h)"), L * H)
            if flags["f2b"]:
                gb_sb["f2b"] = bcast_load("f2b_sb", gb_d["f2b"].rearrange("l h -> (l h)"), L * H)
            if flags["qkv_b"]:
                qkvb_sb = persist.tile([P, L, 3 * H // P], dt.float32, tag="qkvb_sb")
                nc.sync.dma_start(qkvb_sb, gb_d["qkv_b"].rearrange("l (o p) -> p l o", p=P))
                # v bias varies along the free dim in token-major layout
                gb_sb["vb"] = bcast_load("vb_sb", gb_d["qkv_b"].rearrange("l h -> (l h)"), L * 3 * H)
            if flags["f1b"]:
                f1b_sb = persist.tile([P, L, FF // P], dt.float32, tag="f1b_sb")
                nc.sync.dma_start(f1b_sb, gb_d["f1b"].rearrange("l (o p) -> p l o", p=P))

            # ---- layernorm helper: src fp32 [P, H] slice -> dst fp32 [P, H] ----
            def layer_norm(src, dst, g_ap, b_ap):
                stats = work.tile([P, 3, 6], dt.float32, tag="st", name="stats")
                for sg in range(3):
                    nc.vector.bn_stats(stats[:, sg, :], src[:, sg * 256:(sg + 1) * 256])
                mv = work.tile([P, 2], dt.float32, tag="mv", name="mv")
                nc.vector.bn_aggr(mv, stats)
                nc.scalar.activation(mv[:, 1:2], mv[:, 1:2], AF.Sqrt, bias=eps_sb, scale=1.0)
                nc.vector.reciprocal(mv[:, 1:2], mv[:, 1:2])
                nc.vector.tensor_scalar(
                    out=dst,
                    in0=src,
                    scalar1=mv[:, 0:1],
                    scalar2=mv[:, 1:2],
                    op0=OP.subtract,
                    op1=OP.mult,
                )
                if g_ap is not None:
                    nc.vector.tensor_tensor(dst, dst, g_ap, op=OP.mult)
                if b_ap is not None:
                    nc.vector.tensor_tensor(dst, dst, b_ap, op=OP.add)

            # ---- embedding: gather + add + LN -> h ----
            for c in range(NCHUNK):
                idt = work.tile([P, 1], dt.int32, tag="idt", name="idt")
                nc.sync.dma_start(idt, ids_d[c, :, None])
                gat = work.tile([P, H], dt.float32, tag="gat", name="gat")
                import concourse.bass as bass

                nc.gpsimd.indirect_dma_start(
                    out=gat,
                    out_offset=None,
                    in_=wemb_d[:],
                    in_offset=bass.IndirectOffsetOnAxis(ap=idt[:, :1], axis=0),
                )
                sgt = work.tile([P, 1], dt.int32, tag="idt", name="sgt")
                nc.sync.dma_start(sgt, seg_d[c, :, None])
                gat2 = work.tile([P, H], dt.float32, tag="gat", name="gat2")
                nc.gpsimd.indirect_dma_start(
                    out=gat2,
                    out_offset=None,
                    in_=temb_d[:],
                    in_offset=bass.IndirectOffsetOnAxis(ap=sgt[:, :1], axis=0),
                )
                # positions for chunk c: rows c*128 .. c*128+127
                nc.sync.dma_start(t[:, c, :], pemb_d[c * P:(c + 1) * P, :])
                nc.vector.tensor_tensor(h[:, c, :], gat, gat2, op=OP.add)
                nc.vector.tensor_tensor(h[:, c, :], h[:, c, :], t[:, c, :], op=OP.add)
                layer_norm(
                    h[:, c, :],
                    h[:, c, :],
                    gb_sb.get("emb_g"),
                    gb_sb.get("emb_b"),
                )

            # ---- encoder layers ----
            for l in range(L):
                qkvw_l = qkvw_d[l].rearrange("(o p) n -> p o n", p=P)
                ow_l = ow_d[l].rearrange("(o p) n -> p o n", p=P)
                f1w_l = f1w_d[l].rearrange("(o p) n -> p o n", p=P)
                f2w_l = f2w_d[l].rearrange("(o p) n -> p o n", p=P)

                # --- A: hT = transpose(h), cast bf16 ---
                for c in range(NCHUNK):
                    for o in range(KT):
                        tp = ps.tile([P, P], dt.float32, tag="score", name="tp")
                        nc.tensor.transpose(tp, h[:, c, o * P:(o + 1) * P], ident)
                        eng = nc.scalar if (o % 2 == 0) else nc.vector
                        if eng is nc.scalar:
                            nc.scalar.copy(hT[:, o, c * P:(c + 1) * P], tp)
                        else:
                            nc.vector.tensor_copy(hT[:, o, c * P:(c + 1) * P], tp)

                # --- B: qT (scaled), kT, v ---
                for ch in range(3):
                    wq = wpool.tile([P, KT, H], dt.bfloat16, tag="w", name="wq")
                    nc.sync.dma_start(wq, qkvw_l[:, :, ch * H:(ch + 1) * H])
                    if ch < 2:
                        dst = qT if ch == 0 else kTt
                        for oc in range(KT):
                            pt = ps.tile([P, S], dt.float32, tag="acc", name="pt")
                            for kt in range(KT):
                                nc.tensor.matmul(
                                    pt,
                                    lhsT=wq[:, kt, oc * P:(oc + 1) * P],
                                    rhs=hT[:, kt, :],
                                    start=(kt == 0),
                                    stop=(kt == KT - 1),
                                )
                            if ch == 0:
                                if flags["qkv_b"]:
                                    nc.scalar.activation(
                                        dst[:, oc, :], pt, AF.Identity,
                                        bias=qkvb_sb[:, l, oc:oc + 1], scale=1.0 / 8.0,
                                    )
                                else:
                                    nc.scalar.mul(dst[:, oc, :], pt, 1.0 / 8.0)
                            else:
                                if flags["qkv_b"]:
                                    nc.scalar.activation(
                                        dst[:, oc, :], pt, AF.Identity,
                                        bias=qkvb_sb[:, l, KT + oc:KT + oc + 1], scale=1.0,
                                    )
                                else:
                                    nc.vector.tensor_copy(dst[:, oc, :], pt)
                    else:
                        for mc in range(NCHUNK):
                            for n2 in range(2):
                                pt = ps.tile([P, 384], dt.float32, tag="acc", name="ptv")
                                for kt in range(KT):
                                    nc.tensor.matmul(
                                        pt,
                                        lhsT=hT[:, kt, mc * P:(mc + 1) * P],
                                        rhs=wq[:, kt, n2 * 384:(n2 + 1) * 384],
                                        start=(kt == 0),
                                        stop=(kt == KT - 1),
                                    )
                                if flags["qkv_b"]:
                                    vb = gb_sb["vb"]
                                    nc.vector.tensor_tensor(
                                        vtm[:, mc, n2 * 384:(n2 + 1) * 384], pt,
                                        vb[:, l * 3 * H + 2 * H + n2 * 384:l * 3 * H + 2 * H + (n2 + 1) * 384],
                                        op=OP.add,
                                    )
                                else:
                                    nc.vector.tensor_copy(vtm[:, mc, n2 * 384:(n2 + 1) * 384], pt)

                # --- C: attention per head (scoresT orientation) ---
                for hd in range(NH):
                    o = hd // 2
                    r0 = (hd % 2) * HD
                    sc = ps.tile([P, NCHUNK, S], dt.float32, tag="score", name="sc")
                    for kc in range(NCHUNK):
                        nc.tensor.matmul(
                            sc[:, kc, :],
                            lhsT=kTt[r0:r0 + HD, o, kc * P:(kc + 1) * P],
                            rhs=qT[r0:r0 + HD, o, :],
                            start=True,
                            stop=True,
                        )
                    e_sb = work.tile([P, NCHUNK, S], dt.bfloat16, tag="e", name="e_sb")
                    for kc in range(NCHUNK):
                        nc.scalar.activation(
                            e_sb[:, kc, :], sc[:, kc, :], AF.Exp,
                            bias=am_sb[:, kc:kc + 1], scale=1.0,
                        )
                    cs = ps.tile([1, S], dt.float32, tag="sm", name="cs")
                    for kc in range(NCHUNK):
                        nc.tensor.matmul(
                            cs, lhsT=ones_bf, rhs=e_sb[:, kc, :],
                            start=(kc == 0), stop=(kc == NCHUNK - 1),
                        )
                    cx = ps.tile([HD, S], dt.float32, tag="ctx", name="cx")
                    for kc in range(NCHUNK):
                        nc.tensor.matmul(
                            cx,
                            lhsT=vtm[:, kc, hd * HD:(hd + 1) * HD],
                            rhs=e_sb[:, kc, :],
                            start=(kc == 0),
                            stop=(kc == NCHUNK - 1),
                        )
                    rv = work.tile([1, S], dt.float32, tag="rv", name="rv")
                    nc.vector.reciprocal(rv, cs)
                    bc = ps.tile([HD, S], dt.float32, tag="sm", name="bc")
                    nc.tensor.matmul(bc, lhsT=ones1, rhs=rv, start=True, stop=True)
                    bc_sb = work.tile([HD, S], dt.float32, tag="bc_sb", name="bc_sb")
                    nc.scalar.copy(bc_sb, bc)
                    nc.vector.tensor_tensor(ctxT[r0:r0 + HD, o, :], cx, bc_sb, op=OP.mult)

                # --- D: attn out + residual + LN1 ---
                wo = wpool.tile([P, KT, H], dt.bfloat16, tag="w", name="wo")
                nc.sync.dma_start(wo, ow_l)
                for mc in range(NCHUNK):
                    for n2 in range(2):
                        pt = ps.tile([P, 384], dt.float32, tag="acc", name="pta")
                        for kt in range(KT):
                            nc.tensor.matmul(
                                pt,
                                lhsT=ctxT[:, kt, mc * P:(mc + 1) * P],
                                rhs=wo[:, kt, n2 * 384:(n2 + 1) * 384],
                                start=(kt == 0),
                                stop=(kt == KT - 1),
                            )
                        sl = slice(n2 * 384, (n2 + 1) * 384)
                        nc.vector.tensor_tensor(t[:, mc, sl], h[:, mc, sl], pt, op=OP.add)
                        if flags["ob"]:
                            ob = gb_sb["ob"]
                            nc.vector.tensor_tensor(
                                t[:, mc, sl], t[:, mc, sl],
                                ob[:, l * H + n2 * 384:l * H + (n2 + 1) * 384], op=OP.add,
                            )
                for mc in range(NCHUNK):
                    g_ap = gb_sb["ln1_g"][:, l * H:(l + 1) * H] if flags["ln1_gb"] else None
                    b_ap = gb_sb["ln1_b"][:, l * H:(l + 1) * H] if flags["ln1_gb"] else None
                    layer_norm(t[:, mc, :], h2[:, mc, :], g_ap, b_ap)

                # --- E: FFN ---
                for c in range(NCHUNK):
                    for o in range(KT):
                        tp = ps.tile([P, P], dt.float32, tag="score", name="tp2")
                        nc.tensor.transpose(tp, h2[:, c, o * P:(o + 1) * P], ident)
                        if o % 2 == 0:
                            nc.scalar.copy(h2T[:, o, c * P:(c + 1) * P], tp)
                        else:
                            nc.vector.tensor_copy(h2T[:, o, c * P:(c + 1) * P], tp)

                for ch in range(4):
                    w1 = wpool.tile([P, KT, H], dt.bfloat16, tag="w", name="w1")
                    nc.sync.dma_start(w1, f1w_l[:, :, ch * H:(ch + 1) * H])
                    for f_loc in range(KT):
                        oc = ch * KT + f_loc
                        pt = ps.tile([P, S], dt.float32, tag="acc", name="ptf")
                        for kt in range(KT):
                            nc.tensor.matmul(
                                pt,
                                lhsT=w1[:, kt, f_loc * P:(f_loc + 1) * P],
                                rhs=h2T[:, kt, :],
                                start=(kt == 0),
                                stop=(kt == KT - 1),
                            )
                        if flags["f1b"]:
                            nc.scalar.activation(
                                fT[:, oc, :], pt, AF.Gelu,
                                bias=f1b_sb[:, l, oc:oc + 1], scale=1.0,
                            )
                        else:
                            nc.scalar.activation(fT[:, oc, :], pt, AF.Gelu)

                w2 = wbig.tile([P, FKT, H], dt.bfloat16, tag="w2", name="w2")
                nc.sync.dma_start(w2, f2w_l)
                for mc in range(NCHUNK):
                    for n2 in range(2):
                        pt = ps.tile([P, 384], dt.float32, tag="acc", name="pt2")
                        for kt in range(FKT):
                            nc.tensor.matmul(
                                pt,
                                lhsT=fT[:, kt, mc * P:(mc + 1) * P],
                                rhs=w2[:, kt, n2 * 384:(n2 + 1) * 384],
                                start=(kt == 0),
                                stop=(kt == FKT - 1),
                            )
                        sl = slice(n2 * 384, (n2 + 1) * 384)
                        nc.vector.tensor_tensor(t[:, mc, sl], h2[:, mc, sl], pt, op=OP.add)
                        if flags["f2b"]:
                            f2b = gb_sb["f2b"]
                            nc.vector.tensor_tensor(
                                t[:, mc, sl], t[:, mc, sl],
                                f2b[:, l * H + n2 * 384:l * H + (n2 + 1) * 384], op=OP.add,
                            )
                for mc in range(NCHUNK):
                    g_ap = gb_sb["ln2_g"][:, l * H:(l + 1) * H] if flags["ln2_gb"] else None
                    b_ap = gb_sb["ln2_b"][:, l * H:(l + 1) * H] if flags["ln2_gb"] else None
                    layer_norm(t[:, mc, :], h[:, mc, :], g_ap, b_ap)

            # ---- pooling: out = A @ enc  (A is host-precomputed, fp32) ----
            for n2 in range(2):
                pt = ps.tile([P, 384], dt.float32, tag="acc", name="ptp")
                for c in range(NCHUNK):
                    nc.tensor.matmul(
                        pt,
                        lhsT=atp_sb[:, c, :],
                        rhs=h[:, c, n2 * 384:(n2 + 1) * 384],
                        start=(c == 0),
                        stop=(c == NCHUNK - 1),
                    )
                nc.vector.tensor_copy(out_sb[:, n2 * 384:(n2 + 1) * 384], pt)
            nc.sync.dma_start(out_d, out_sb)

    nc.finalize()
    return nc


def _host_prep(inputs):
    """Build per-core in_maps from the full inputs."""
    ids = np.asarray(inputs["bert_ids"])
    segs = np.asarray(inputs["segments"])
    mask = np.asarray(inputs["bert_mask"])
    lens = np.asarray(inputs["bert_lens"])
    f32 = lambda k: np.asarray(inputs[k], dtype=np.float32)
    word_emb = f32("word_emb")
    pos_emb = f32("pos_emb")[:S]
    type_emb = f32("type_emb")

    flags = {
        "emb_gb": not (
            np.all(np.asarray(inputs["emb_ln_g"]) == 1.0)
            and np.all(np.asarray(inputs["emb_ln_b"]) == 0.0)
        ),
        "ln1_gb": not (
            np.all(np.asarray(inputs["ln1_g"]) == 1.0)
            and np.all(np.asarray(inputs["ln1_b"]) == 0.0)
        ),
        "ln2_gb": not (
            np.all(np.asarray(inputs["ln2_g"]) == 1.0)
            and np.all(np.asarray(inputs["ln2_b"]) == 0.0)
        ),
        "qkv_b": bool(np.any(np.asarray(inputs["qkv_b"]) != 0.0)),
        "ob": bool(np.any(np.asarray(inputs["attn_out_b"]) != 0.0)),
        "f1b": bool(np.any(np.asarray(inputs["ff1_b"]) != 0.0)),
        "f2b": bool(np.any(np.asarray(inputs["ff2_b"]) != 0.0)),
    }

    shared = {
        "word_emb": word_emb,
        "pos_emb": np.ascontiguousarray(pos_emb),
        "type_emb": type_emb,
        "qkv_w": np.asarray(inputs["qkv_w"]).astype(BF16),
        "attn_out_w": np.asarray(inputs["attn_out_w"]).astype(BF16),
        "ff1_w": np.asarray(inputs["ff1_w"]).astype(BF16),
        "ff2_w": np.asarray(inputs["ff2_w"]).astype(BF16),
    }
    if flags["emb_gb"]:
        shared["emb_g"] = f32("emb_ln_g")
        shared["emb_b"] = f32("emb_ln_b")
    if flags["ln1_gb"]:
        shared["ln1_g"] = f32("ln1_g")
        shared["ln1_b"] = f32("ln1_b")
    if flags["ln2_gb"]:
        shared["ln2_g"] = f32("ln2_g")
        shared["ln2_b"] = f32("ln2_b")
    if flags["qkv_b"]:
        qb = f32("qkv_b").copy()
        qb[:, :H] *= 1.0 / 8.0  # fold the q 1/sqrt(HD) scale into the bias
        shared["qkv_b"] = qb
    if flags["ob"]:
        shared["ob"] = f32("attn_out_b")
    if flags["f1b"]:
        shared["f1b"] = f32("ff1_b")
    if flags["f2b"]:
        shared["f2b"] = f32("ff2_b")

    in_maps = []
    for c in range(BZ):
        m = mask[c].astype(np.int64)
        ln = lens[c].astype(np.int64)
        total = int(ln.sum())
        # token index per subword position (prefix mask => rank == position)
        bounds = np.cumsum(ln)
        tok = np.searchsorted(bounds, np.arange(S), side="right")
        tok = np.minimum(tok, T - 1)
        A = np.zeros((T, S), dtype=np.float32)
        for s_i in range(min(total, S)):
            t_i = tok[s_i]
            A[t_i, s_i] = 1.0 / max(int(ln[t_i]), 1)
        atp = np.ascontiguousarray(A.T.reshape(NCHUNK, P, T).transpose(1, 0, 2))
        am = ((1.0 - m.astype(np.float32)) * -10000.0).reshape(NCHUNK, P).T
        im = {
            "ids32": ids[c].reshape(NCHUNK, P).astype(np.int32),
            "seg32": segs[c].reshape(NCHUNK, P).astype(np.int32),
            "am": np.ascontiguousarray(am),
            "atp": atp,
        }
        im.update(shared)
        in_maps.append(im)
    return in_maps, flags


def _run(inputs, trace=False):
    from concourse import bass_utils

    in_maps, flags = _host_prep(inputs)
    key = tuple(sorted(flags.items()))
    if key not in _CACHE:
        _CACHE[key] = _build_program(flags)
    nc = _CACHE[key]
    res = bass_utils.run_bass_kernel_spmd(
        nc, in_maps, core_ids=list(range(BZ)), trace=trace
    )
    out = np.stack([np.asarray(res.results[c]["outp"]) for c in range(BZ)], axis=0)
    return out.astype(np.float32), res


def kernel(**inputs):
    out, _ = _run(inputs, trace=False)
    return out


def bench(inputs, iters=10):
    """Time kernel execution with device-resident inputs (excludes H2D of
    weights). Returns (min_s, all_times, outputs_core0_check)."""
    import time

    import jax
    import jax.numpy as jnp
    from jax.sharding import Mesh, NamedSharding, PartitionSpec

    try:
        from jax.experimental.shard_map import shard_map
    except ImportError:
        from jax.shard_map import shard_map
    import concourse.mybir as mybir
    from concourse import bass2jax

    in_maps, flags = _host_prep(inputs)
    key = tuple(sorted(flags.items()))
    if key not in _CACHE:
        _CACHE[key] = _build_program(flags)
    nc = _CACHE[key]
    n_cores = BZ

    bass2jax.install_neuronx_cc_hook()
    partition_name = nc.partition_id_tensor.name if nc.partition_id_tensor else None

    in_names, out_names, out_avals, zero_shapes = [], [], [], []
    for alloc in nc.m.functions[0].allocations:
        if not isinstance(alloc, mybir.MemoryLocationSet):
            continue
        name = alloc.memorylocations[0].name
        if alloc.kind == "ExternalInput":
            if name != partition_name:
                in_names.append(name)
        elif alloc.kind == "ExternalOutput":
            shape = tuple(alloc.tensor_shape)
            dtype = mybir.dt.np(alloc.dtype)
            out_names.append(name)
            out_avals.append(jax.core.ShapedArray(shape, dtype))
            zero_shapes.append((shape, dtype))
    n_params = len(in_names)
    all_names = list(in_names) + list(out_names)
    if partition_name is not None:
        all_names.append(partition_name)
    donate = tuple(range(n_params, n_params + len(out_names)))

    def _body(*args):
        operands = list(args)
        if partition_name is not None:
            operands.append(bass2jax.partition_id_tensor())
        outs = bass2jax._bass_exec_p.bind(
            *operands,
            out_avals=tuple(out_avals),
            in_names=tuple(all_names),
            out_names=tuple(out_names),
            lowering_input_output_aliases=(),
            sim_require_finite=True,
            sim_require_nnan=True,
            nc=nc,
        )
        return tuple(outs)

    devices = jax.devices()[:n_cores]
    mesh = Mesh(np.asarray(devices), ("core",))
    spec = PartitionSpec("core")
    sharded = jax.jit(
        shard_map(
            _body,
            mesh=mesh,
            in_specs=(spec,) * (n_params + len(out_names)),
            out_specs=(spec,) * len(out_names),
            check_rep=False,
        ),
        donate_argnums=donate,
        keep_unused=True,
    )
    shd = NamedSharding(mesh, spec)
    concat_in = [
        jax.device_put(
            np.concatenate([np.asarray(in_maps[c][nm]) for c in range(n_cores)], axis=0),
            shd,
        )
        for nm in in_names
    ]
    jax.block_until_ready(concat_in)

    def fresh_zeros():
        return [
            jax.device_put(np.zeros((n_cores * s[0], *s[1:]), d), shd)
            for (s, d) in zero_shapes
        ]

    times = []
    out = None
    for i in range(iters):
        z = fresh_zeros()
        jax.block_until_ready(z)
        t0 = time.perf_counter()
        out = sharded(*concat_in, *z)
        jax.block_until_ready(out)
        times.append(time.perf_counter() - t0)
    return min(times), times, np.asarray(out[0])
